# revision 1
# baseline (speedup 1.0000x reference)
"""GATv2 backbone (4 layers) on 8 Trainium2 NeuronCores.

Strategy:
  * Nodes partitioned into 8 contiguous ranges (edge-balanced). Edges owned by
    the core owning their dst node; sorted by dst, grouped into 128-node
    windows, padded to 128-edge tiles (tile count per window uniform across
    cores, baked into the program at build time).
  * Per layer: xl = h @ Wl computed on the local shard, AllGathered into a
    full DRAM table; xl[src] fetched per edge tile via indirect DMA.
    xr contribution expanded from the window's 128 rows via a one-hot S
    matmul; ef = ew*We via rank-1 matmul; all accumulated in PSUM.
  * Softmax denominators and the weighted scatter are matmuls against the
    one-hot S^T built on-chip with iota + is_equal (padded edges get dst=-1
    so their one-hot column is zero).
  * Graph-LayerNorm stats via per-window node->graph one-hot matmuls
    accumulated in PSUM, AllReduced across cores (1x100 floats).
"""

import contextlib

import ml_dtypes
import numpy as np

from concourse import bass, bacc, mybir, tile
from concourse.bass_utils import run_bass_kernel_spmd
from concourse.masks import make_identity

P = 128
NCORES = 8
GMAX = 50          # graphs
HEADS = 4
DHID = 128
CH = DHID // HEADS          # 32
DF = 512                    # final per-head concat width (4*128)
NEG = 0.2
EPS = 1e-5
USE_LRELU = False           # leaky via x + relu(-0.8 x): sim == hw

F32 = mybir.dt.float32
BF = mybir.dt.bfloat16
I32 = mybir.dt.int32
AX = mybir.AxisListType
OP = mybir.AluOpType
AF = mybir.ActivationFunctionType


# ----------------------------------------------------------------------------
# Host preprocessing: graph partitioning + static schedule
# ----------------------------------------------------------------------------

def build_meta(edge_index, batch):
    N = batch.shape[0]
    E = edge_index.shape[1]
    src = np.asarray(edge_index[0], dtype=np.int64)
    dst = np.asarray(edge_index[1], dtype=np.int64)
    batch = np.asarray(batch, dtype=np.int64)

    deg = np.bincount(dst, minlength=N)
    cum = np.concatenate([[0], np.cumsum(deg)])      # edges with dst < n

    bounds = [0]
    for c in range(1, NCORES):
        n = int(np.searchsorted(cum, c * E / NCORES))
        bounds.append(min(max(n, bounds[-1] + 1), N - (NCORES - c)))
    bounds.append(N)
    lo = np.array(bounds[:-1])
    hi = np.array(bounds[1:])

    NW = int(max((hi - lo + P - 1) // P))
    NPAD = NW * P
    NTOT = NCORES * NPAD

    # node -> table row in the allgathered layout
    trow = np.zeros(N, np.int64)
    for c in range(NCORES):
        trow[lo[c]:hi[c]] = c * NPAD + np.arange(hi[c] - lo[c])

    order = np.argsort(dst, kind="stable")
    src_s = src[order]

    # per (core, window) edge slices out of the dst-sorted list
    cnt = np.zeros((NCORES, NW), np.int64)
    sl = {}
    for c in range(NCORES):
        for w in range(NW):
            a = lo[c] + w * P
            b = min(a + P, hi[c])
            if a >= b:
                sl[(c, w)] = (0, 0)
                continue
            e0, e1 = int(cum[a]), int(cum[b])
            sl[(c, w)] = (e0, e1)
            cnt[c, w] = e1 - e0

    Tw = np.maximum(1, (cnt.max(axis=0) + P - 1) // P).astype(np.int64)
    toff = np.concatenate([[0], np.cumsum(Tw)])      # tile offset per window
    TT = int(toff[-1])

    gidx = np.zeros((NCORES, P, TT), np.int32)       # table row of src (pad->0)
    dsti = np.full((NCORES, P, TT), -1, np.int32)    # dst local in window
    dstf = np.full((NCORES, TT * P), -1.0, np.float32)
    ewsl = np.full((NCORES, TT * P), -1, np.int64)   # edge id per slot (-1 pad)
    for c in range(NCORES):
        for w in range(NW):
            e0, e1 = sl[(c, w)]
            n = e1 - e0
            if n == 0:
                continue
            ids = order[e0:e1]
            ids = ids[np.argsort(src_s[e0:e1], kind="stable")]   # src locality
            t0 = int(toff[w])
            slot = np.arange(n)
            tt = t0 + slot // P
            pp = slot % P
            gidx[c, pp, tt] = trow[src[ids]]
            dl = (dst[ids] - (lo[c] + w * P)).astype(np.int32)
            dsti[c, pp, tt] = dl
            flat = tt * P + pp
            dstf[c, flat] = dl.astype(np.float32)
            ewsl[c, flat] = ids

    # graph one-hots per (core, window): rows beyond hi are all-zero
    gmat = np.zeros((NCORES, NW, P, GMAX), np.float32)
    for c in range(NCORES):
        nreal = int(hi[c] - lo[c])
        g = batch[lo[c]:hi[c]]
        r = np.arange(nreal)
        gmat[c, r // P, r % P, g] = 1.0
    gmatT = np.ascontiguousarray(np.swapaxes(gmat, 2, 3))

    cntg = np.bincount(batch, minlength=GMAX).astype(np.float32)
    invd = (1.0 / (np.maximum(cntg, 1.0) * DHID)).reshape(1, GMAX)

    sel4 = np.zeros((HEADS, P), np.float32)
    for h in range(HEADS):
        sel4[h, h * CH:(h + 1) * CH] = 1.0

    return dict(N=N, E=E, NW=NW, NPAD=NPAD, NTOT=NTOT, TT=TT,
                Tw=Tw.astype(int), toff=toff.astype(int), lo=lo, hi=hi,
                gidx=gidx, dsti=dsti, dstf=dstf, ewsl=ewsl,
                gmat=gmat, gmatT=gmatT, invd=invd, sel4=sel4)


# ----------------------------------------------------------------------------
# Bass program
# ----------------------------------------------------------------------------

def build_program(meta):
    NW, NPAD, NTOT, TT = meta["NW"], meta["NPAD"], meta["NTOT"], meta["TT"]
    Tw, toff = meta["Tw"], meta["toff"]

    nc = bacc.Bacc("TRN2", target_bir_lowering=False, debug=False,
                   enable_asserts=False, num_devices=NCORES)

    # --- external I/O (per core) ---
    h0s = nc.dram_tensor("h0s", [NPAD, P], BF, kind="ExternalInput")
    rs = nc.dram_tensor("rs", [NPAD, P], F32, kind="ExternalInput")
    gidx_d = nc.dram_tensor("gidx", [P, TT], I32, kind="ExternalInput")
    dsti_d = nc.dram_tensor("dsti", [P, TT], I32, kind="ExternalInput")
    dstf_d = nc.dram_tensor("dstf", [1, TT * P], F32, kind="ExternalInput")
    ew_d = nc.dram_tensor("ew", [1, TT * P], BF, kind="ExternalInput")
    gmat_d = nc.dram_tensor("gmat", [NW, P, GMAX], F32, kind="ExternalInput")
    gmatT_d = nc.dram_tensor("gmatT", [NW, GMAX, P], F32, kind="ExternalInput")
    invd_d = nc.dram_tensor("invd", [1, GMAX], F32, kind="ExternalInput")
    sel4_d = nc.dram_tensor("sel4", [HEADS, P], F32, kind="ExternalInput")

    wl_d = nc.dram_tensor("wl", [3, P, P], BF, kind="ExternalInput")
    wr_d = nc.dram_tensor("wr", [3, P, P], BF, kind="ExternalInput")
    blr_d = nc.dram_tensor("blr", [3, P, P], F32, kind="ExternalInput")
    brr_d = nc.dram_tensor("brr", [3, P, P], F32, kind="ExternalInput")
    attr_d = nc.dram_tensor("attr", [3, P, P], F32, kind="ExternalInput")
    we_d = nc.dram_tensor("we", [3, 1, P], BF, kind="ExternalInput")
    lnw_d = nc.dram_tensor("lnw", [3, P, P], F32, kind="ExternalInput")
    lnb_d = nc.dram_tensor("lnb", [3, P, P], F32, kind="ExternalInput")
    bia_d = nc.dram_tensor("bia", [3, P, P], F32, kind="ExternalInput")

    wlf_d = nc.dram_tensor("wlf", [P, DF], BF, kind="ExternalInput")
    wrf_d = nc.dram_tensor("wrf", [P, DF], BF, kind="ExternalInput")
    blfr_d = nc.dram_tensor("blfr", [P, DF], F32, kind="ExternalInput")
    brfr_d = nc.dram_tensor("brfr", [P, DF], F32, kind="ExternalInput")
    attfr_d = nc.dram_tensor("attfr", [P, DF], F32, kind="ExternalInput")
    wef_d = nc.dram_tensor("wef", [1, DF], BF, kind="ExternalInput")
    biafr_d = nc.dram_tensor("biafr", [P, P], F32, kind="ExternalInput")

    out_d = nc.dram_tensor("out", [NPAD, P], F32, kind="ExternalOutput")

    with tile.TileContext(nc) as tc, contextlib.ExitStack() as ctx:
        dram = ctx.enter_context(tc.tile_pool(name="dram", bufs=1, space="DRAM"))
        cst = ctx.enter_context(tc.tile_pool(name="cst", bufs=1))
        per = ctx.enter_context(tc.tile_pool(name="per", bufs=1))
        wsp = ctx.enter_context(tc.tile_pool(name="wsp", bufs=2))
        gpo = ctx.enter_context(tc.tile_pool(name="gpo", bufs=6))

        xl_b = dram.tile([NPAD, P], BF)
        xl_full = dram.tile([NTOT, P], BF)
        xlf_b = dram.tile([NPAD, DF], BF)
        xlf_full = dram.tile([NTOT, DF], BF)
        st_b = dram.tile([2, GMAX], F32)
        st_o = dram.tile([2, GMAX], F32)
        groups = [list(range(NCORES))]

        # --- constants ---
        ident = cst.tile([P, P], F32)
        make_identity(nc, ident[:])
        identb = cst.tile([P, P], BF)
        nc.vector.tensor_copy(out=identb[:], in_=ident[:])
        iota_row = cst.tile([P, P], I32)
        nc.gpsimd.iota(iota_row[:], pattern=[[1, P]], base=0, channel_multiplier=0)
        iota_ci = cst.tile([P, 1], I32)
        nc.gpsimd.iota(iota_ci[:], pattern=[[1, 1]], base=0, channel_multiplier=1)
        iota_cf = cst.tile([P, 1], F32)
        nc.vector.tensor_copy(out=iota_cf[:], in_=iota_ci[:])
        ones1 = cst.tile([1, P], F32)
        nc.vector.memset(ones1[:], 1.0)
        epsc = cst.tile([P, 1], F32)
        nc.vector.memset(epsc[:], EPS)
        sel4 = cst.tile([HEADS, P], F32)
        nc.sync.dma_start(out=sel4[:], in_=sel4_d[:, :])
        invd = cst.tile([1, GMAX], F32)
        nc.sync.dma_start(out=invd[:], in_=invd_d[:, :])
        gidx_s = cst.tile([P, TT], I32)
        nc.sync.dma_start(out=gidx_s[:], in_=gidx_d[:, :])
        dsti_s = cst.tile([P, TT], I32)
        nc.sync.dma_start(out=dsti_s[:], in_=dsti_d[:, :])

        # persistent per-layer node-state (window-major)
        per_kw = dict(tag="", bufs=1)
        h_a = per.tile([P, NW, P], BF, tag="h_a")
        hT = per.tile([P, NW, P], BF, tag="hT")
        htmp = per.tile([P, NW, P], F32, tag="htmp")

        for w in range(NW):
            nc.sync.dma_start(out=h_a[:, w, :], in_=h0s[w * P:(w + 1) * P, :])

        def leaky(dst_ap, src_ap, shape):
            if USE_LRELU:
                nc.scalar.activation(out=dst_ap, in_=src_ap, func=AF.Lrelu,
                                     alpha=NEG)
            else:
                r = wsp.tile(shape, F32, tag="lrtmp", bufs=1, name="lr")
                rr = r[tuple(slice(0, s) for s in dst_ap.shape)]
                nc.scalar.activation(out=rr, in_=src_ap, func=AF.Relu,
                                     scale=-(1.0 - NEG))
                nc.vector.tensor_tensor(out=dst_ap, in0=src_ap, in1=rr,
                                        op=OP.add)

        # ------------------------------------------------------------------
        def hidden_layer(li, h_cur, h_nxt, add_resid):
            wl = cst.tile([P, P], BF, tag="wlc", name="wl_t")
            nc.sync.dma_start(out=wl[:], in_=wl_d[li])
            wr = cst.tile([P, P], BF, tag="wrc", name="wr_t")
            nc.sync.dma_start(out=wr[:], in_=wr_d[li])
            blr = cst.tile([P, P], F32, tag="blrc", name="blr_t")
            nc.sync.dma_start(out=blr[:], in_=blr_d[li])
            brr = cst.tile([P, P], F32, tag="brrc", name="brr_t")
            nc.sync.dma_start(out=brr[:], in_=brr_d[li])
            attr = cst.tile([P, P], F32, tag="attrc", name="attr_t")
            nc.sync.dma_start(out=attr[:], in_=attr_d[li])
            wer = cst.tile([1, P], BF, tag="werc", name="wer_t")
            nc.sync.dma_start(out=wer[:], in_=we_d[li])
            lnw = cst.tile([P, P], F32, tag="lnwc", name="lnw_t")
            nc.sync.dma_start(out=lnw[:], in_=lnw_d[li])
            lnb = cst.tile([P, P], F32, tag="lnbc", name="lnb_t")
            nc.sync.dma_start(out=lnb[:], in_=lnb_d[li])
            bia = cst.tile([P, P], F32, tag="biac", name="bia_t")
            nc.sync.dma_start(out=bia[:], in_=bia_d[li])

            with tc.tile_pool(name=f"ps{li}", bufs=1, space="PSUM") as ps:
                # PSUM budget: ep(2) + db(1) + nmr(1) + dnm(1) + stats(1)
                #            + pt(1) + px(1) = 8 banks
                def ep_t():
                    return ps.tile([P, 4 * P], F32, space="PSUM", tag="ep",
                                   bufs=2, name="ep")

                def db_t():
                    return ps.tile([P, 4 * P], F32, space="PSUM", tag="db",
                                   name="db")

                def pt_t():
                    return ps.tile([P, P], F32, space="PSUM", tag="pt",
                                   name="pt")

                def px_t():
                    return ps.tile([P, P], F32, space="PSUM", tag="px",
                                   name="px")

                # P0: transposes + xl shard -> DRAM bounce
                for w in range(NW):
                    tp = ps.tile([P, P], BF, space="PSUM", tag="pt",
                                 name="ptb")
                    nc.tensor.transpose(out=tp[:], in_=h_cur[:, w, :],
                                        identity=identb[:])
                    nc.vector.tensor_copy(out=hT[:, w, :], in_=tp[:])
                    xp = px_t()
                    nc.tensor.matmul(out=xp[:], lhsT=hT[:, w, :], rhs=wl[:],
                                     start=True, stop=True)
                    xs = wsp.tile([P, P], BF, tag="p0xs", name="xs")
                    nc.vector.tensor_tensor(out=xs[:], in0=xp[:], in1=blr[:],
                                            op=OP.add)
                    nc.sync.dma_start(out=xl_b[w * P:(w + 1) * P, :], in_=xs[:])

                # P1: AllGather xl
                nc.gpsimd.collective_compute(
                    "AllGather", OP.bypass, replica_groups=groups,
                    ins=[xl_b.opt()], outs=[xl_full.opt()])

                # P2: edge pipeline per window
                stp = ps.tile([2, GMAX], F32, space="PSUM", tag="stats",
                              name="stp")
                for w in range(NW):
                    T = int(Tw[w])
                    t0 = int(toff[w])
                    xrp = px_t()
                    nc.tensor.matmul(out=xrp[:], lhsT=hT[:, w, :], rhs=wr[:],
                                     start=True, stop=True)
                    xr = wsp.tile([P, P], BF, tag="xr", name="xr")
                    nc.vector.tensor_tensor(out=xr[:], in0=xrp[:], in1=brr[:],
                                            op=OP.add)
                    nmr = ps.tile([P, P], F32, space="PSUM", tag="nmr",
                                  name="nmr")
                    dnm = ps.tile([P, HEADS], F32, space="PSUM", tag="dnm",
                                  name="dnm")

                    nq = (T + 3) // 4
                    for q in range(nq):
                        Q = min(4, T - q * 4)
                        ts = q * 4
                        ep = ep_t()
                        db = db_t()
                        dstf_s = wsp.tile([1, 4 * P], F32, tag="dstf", name="dsf")
                        nc.sync.dma_start(
                            out=dstf_s[0:1, :Q * P],
                            in_=dstf_d[0:1, (t0 + ts) * P:(t0 + ts + Q) * P])
                        ew_s = wsp.tile([1, 4 * P], BF, tag="ews", name="ews")
                        nc.sync.dma_start(
                            out=ew_s[0:1, :Q * P],
                            in_=ew_d[0:1, (t0 + ts) * P:(t0 + ts + Q) * P])
                        gq = gpo.tile([P, 4, P], BF, tag="gq", name="gq")
                        for t in range(Q):
                            nc.gpsimd.indirect_dma_start(
                                out=gq[:, t, :], out_offset=None,
                                in_=xl_full.opt(),
                                in_offset=bass.IndirectOffsetOnAxis(
                                    ap=gidx_s[:, t0 + ts + t:t0 + ts + t + 1],
                                    axis=0))
                        nc.tensor.matmul(
                            out=db[:, :Q * P], lhsT=ones1[:],
                            rhs=dstf_s[0:1, :Q * P],
                            start=True, stop=True)
                        S = wsp.tile([P, 4 * P], BF, tag="S", name="S")
                        nc.vector.tensor_tensor(
                            out=S[:, :Q * P], in0=db[:, :Q * P],
                            in1=iota_cf[:, 0:1].to_broadcast([P, Q * P]),
                            op=OP.is_equal)
                        ST = wsp.tile([P, 4, P], BF, tag="ST", name="ST")
                        nc.vector.tensor_tensor(
                            out=ST[:, :Q, :],
                            in0=iota_row[:, None, :].to_broadcast([P, Q, P]),
                            in1=dsti_s[:, t0 + ts:t0 + ts + Q, None]
                                .to_broadcast([P, Q, P]),
                            op=OP.is_equal)
                        # e_pre = xl[src] + S@xr + ew*We  (PSUM accumulate;
                        # one start/stop group per PSUM bank)
                        nc.tensor.matmul(out=ep[:, :Q * P], lhsT=identb[:],
                                         rhs=gq[:, :Q, :], start=True,
                                         stop=False)
                        for t in range(Q):
                            blk = ep[:, t * P:(t + 1) * P]
                            nc.tensor.matmul(out=blk,
                                             lhsT=S[:, t * P:(t + 1) * P],
                                             rhs=xr[:], start=False, stop=False)
                            nc.tensor.matmul(
                                out=blk,
                                lhsT=ew_s[0:1, t * P:(t + 1) * P],
                                rhs=wer[:], start=False, stop=(t == Q - 1))
                        ea = wsp.tile([P, 4 * P], F32, tag="ea", name="ea")
                        leaky(ea[:, :Q * P], ep[:, :Q * P], [P, 4 * P])
                        lg = wsp.tile([P, 4 * P], F32, tag="lg", name="lg")
                        nc.vector.tensor_tensor(
                            out=lg[:, :Q * P], in0=ea[:, :Q * P],
                            in1=attr[:, None, :].to_broadcast([P, Q, P]),
                            op=OP.mult)
                        lgr = wsp.tile([P, 4 * HEADS], F32, tag="lgr",
                                       name="lgr")
                        nc.vector.tensor_reduce(
                            out=lgr[:, :Q * HEADS],
                            in_=lg[:].rearrange("p (t h c) -> p (t h) c",
                                                h=HEADS, c=CH)[:, :Q * HEADS, :],
                            axis=AX.X, op=OP.add)
                        wq = wsp.tile([P, 4 * HEADS], BF, tag="wq", name="wq")
                        nc.scalar.activation(out=wq[:, :Q * HEADS],
                                             in_=lgr[:, :Q * HEADS], func=AF.Exp)
                        mm = wsp.tile([P, 4, HEADS, CH], BF, tag="mm",
                                      name="mmt")
                        nc.vector.tensor_tensor(
                            out=mm[:, :Q, :, :],
                            in0=gq[:].rearrange("p t (h c) -> p t h c",
                                                h=HEADS, c=CH)[:, :Q, :, :],
                            in1=wq[:].rearrange("p (t h) -> p t h", h=HEADS)
                                [:, :Q, :, None].to_broadcast([P, Q, HEADS, CH]),
                            op=OP.mult)
                        for t in range(Q):
                            first = (q == 0 and t == 0)
                            last = (q == nq - 1 and t == Q - 1)
                            nc.tensor.matmul(
                                out=nmr[:], lhsT=ST[:, t, :], rhs=mm[:, t, :, :],
                                start=first, stop=last)
                            nc.tensor.matmul(
                                out=dnm[:], lhsT=ST[:, t, :],
                                rhs=wq[:, t * HEADS:(t + 1) * HEADS],
                                start=first, stop=last)

                    # window flush (node-major, no transposes)
                    rd = wsp.tile([P, HEADS], F32, tag="rd", name="rd")
                    nc.vector.tensor_scalar(out=rd[:], in0=dnm[:],
                                            scalar1=1e-16, scalar2=None,
                                            op0=OP.add)
                    nc.vector.reciprocal(out=rd[:], in_=rd[:])
                    oT = wsp.tile([P, HEADS, CH], F32, tag="oT", name="oT")
                    nc.vector.tensor_tensor(
                        out=oT[:],
                        in0=nmr[:].rearrange("p (h c) -> p h c", h=HEADS, c=CH),
                        in1=rd[:, :, None].to_broadcast([P, HEADS, CH]),
                        op=OP.mult)
                    nc.vector.tensor_tensor(
                        out=htmp[:, w, :],
                        in0=oT[:].rearrange("p h c -> p (h c)"),
                        in1=bia[:], op=OP.add)
                    # stats: [row-sum | row-sumsq] -> per-graph (PSUM accum)
                    s12 = wsp.tile([P, 2], F32, tag="s12", name="s12")
                    nc.vector.tensor_reduce(out=s12[:, 0:1], in_=htmp[:, w, :],
                                            axis=AX.X, op=OP.add)
                    sqj = wsp.tile([P, P], F32, tag="sqj", name="sqj")
                    nc.scalar.activation(out=sqj[:], in_=htmp[:, w, :],
                                         func=AF.Square, accum_out=s12[:, 1:2])
                    gm = wsp.tile([P, GMAX], F32, tag="gm", name="gm")
                    nc.sync.dma_start(out=gm[:], in_=gmat_d[w])
                    nc.tensor.matmul(out=stp[:, :], lhsT=s12[:],
                                     rhs=gm[:], start=(w == 0),
                                     stop=(w == NW - 1))

                # P3: stats -> mean/rstd -> normalize + elu
                sts = wsp.tile([2, GMAX], F32, tag="sts", name="sts")
                nc.vector.tensor_copy(out=sts[:], in_=stp[:])
                nc.sync.dma_start(out=st_b[:, :], in_=sts[:])
                nc.gpsimd.collective_compute(
                    "AllReduce", OP.add, replica_groups=groups,
                    ins=[st_b.opt()], outs=[st_o.opt()])
                stg1 = wsp.tile([1, GMAX], F32, tag="stg1", name="stg1")
                nc.sync.dma_start(out=stg1[:], in_=st_o[0:1, :])
                stg2 = wsp.tile([1, GMAX], F32, tag="stg2", name="stg2")
                nc.sync.dma_start(out=stg2[:], in_=st_o[1:2, :])
                mean = wsp.tile([1, GMAX], F32, tag="mean", name="mean")
                nc.vector.tensor_tensor(out=mean[:], in0=stg1[:],
                                        in1=invd[:], op=OP.mult)
                ex2 = wsp.tile([1, GMAX], F32, tag="ex2", name="ex2")
                nc.vector.tensor_tensor(out=ex2[:], in0=stg2[:],
                                        in1=invd[:], op=OP.mult)
                msq = wsp.tile([1, GMAX], F32, tag="msq", name="msq")
                nc.scalar.activation(out=msq[:], in_=mean[:], func=AF.Square)
                var = wsp.tile([1, GMAX], F32, tag="var", name="var")
                nc.vector.tensor_tensor(out=var[:], in0=ex2[:], in1=msq[:],
                                        op=OP.subtract)
                sd = wsp.tile([1, GMAX], F32, tag="sd", name="sd")
                nc.scalar.activation(out=sd[:], in_=var[:], func=AF.Sqrt,
                                     bias=epsc[0:1, 0:1])
                rstd = wsp.tile([1, GMAX], F32, tag="rstd", name="rstd")
                nc.vector.reciprocal(out=rstd[:], in_=sd[:])
                nmr2 = wsp.tile([1, GMAX], F32, tag="nmr2", name="nm2")
                nc.vector.tensor_tensor(out=nmr2[:], in0=mean[:], in1=rstd[:],
                                        op=OP.mult)
                nc.vector.tensor_scalar(out=nmr2[:], in0=nmr2[:], scalar1=-1.0,
                                        scalar2=None, op0=OP.mult)
                t1 = pt_t()
                nc.tensor.transpose(out=t1[0:GMAX, 0:1], in_=nmr2[:],
                                    identity=ident[0:1, 0:1])
                t2 = px_t()
                nc.tensor.transpose(out=t2[0:GMAX, 0:1], in_=rstd[:],
                                    identity=ident[0:1, 0:1])
                nrcol = wsp.tile([GMAX, 2], F32, tag="nrcol", name="nrc")
                nc.vector.tensor_copy(out=nrcol[:, 0:1], in_=t1[0:GMAX, 0:1])
                nc.vector.tensor_copy(out=nrcol[:, 1:2], in_=t2[0:GMAX, 0:1])

                for w in range(NW):
                    gmT = wsp.tile([GMAX, P], F32, tag="gmT", name="gmT")
                    nc.sync.dma_start(out=gmT[:], in_=gmatT_d[w])
                    mw = pt_t()
                    nc.tensor.matmul(out=mw[:, 0:2], lhsT=gmT[:], rhs=nrcol[:],
                                     start=True, stop=True)
                    mws = wsp.tile([P, 2], F32, tag="mws", name="mws")
                    nc.vector.tensor_copy(out=mws[:], in_=mw[:, 0:2])
                    xn = wsp.tile([P, P], F32, tag="xn", name="xn")
                    nc.scalar.activation(out=xn[:], in_=htmp[:, w, :],
                                         func=AF.Identity, scale=mws[:, 1:2],
                                         bias=mws[:, 0:1])
                    nc.vector.tensor_tensor(out=xn[:], in0=xn[:], in1=lnw[:],
                                            op=OP.mult)
                    nc.vector.tensor_tensor(out=xn[:], in0=xn[:], in1=lnb[:],
                                            op=OP.add)
                    # elu = max(x,0) + exp(min(x,0)) - 1
                    mn = wsp.tile([P, P], F32, tag="mn", name="mn")
                    nc.vector.tensor_scalar(out=mn[:], in0=xn[:], scalar1=0.0,
                                            scalar2=None, op0=OP.min)
                    nc.scalar.activation(out=mn[:], in_=mn[:], func=AF.Exp)
                    mx = wsp.tile([P, P], F32, tag="mx", name="mx")
                    nc.vector.tensor_scalar(out=mx[:], in0=xn[:], scalar1=0.0,
                                            scalar2=None, op0=OP.max)
                    nc.vector.tensor_tensor(out=mx[:], in0=mx[:], in1=mn[:],
                                            op=OP.add)
                    if add_resid:
                        nc.vector.tensor_scalar(out=mx[:], in0=mx[:],
                                                scalar1=1.0, scalar2=None,
                                                op0=OP.subtract)
                        rt = wsp.tile([P, P], F32, tag="rt", name="rt")
                        nc.sync.dma_start(out=rt[:],
                                          in_=rs[w * P:(w + 1) * P, :])
                        nc.vector.tensor_tensor(out=h_nxt[:, w, :], in0=mx[:],
                                                in1=rt[:], op=OP.add)
                    else:
                        nc.vector.tensor_scalar(out=h_nxt[:, w, :], in0=mx[:],
                                                scalar1=1.0, scalar2=None,
                                                op0=OP.subtract)

        # ------------------------------------------------------------------
        def final_layer(h_cur):
            wlf = cst.tile([P, DF], BF, tag="wlf", name="wlf_t")
            nc.sync.dma_start(out=wlf[:], in_=wlf_d[:, :])
            wrf = cst.tile([P, DF], BF, tag="wrf", name="wrf_t")
            nc.sync.dma_start(out=wrf[:], in_=wrf_d[:, :])
            blfr = cst.tile([P, DF], F32, tag="blfr", name="blf_t")
            nc.sync.dma_start(out=blfr[:], in_=blfr_d[:, :])
            brfr = cst.tile([P, DF], F32, tag="brfr", name="brf_t")
            nc.sync.dma_start(out=brfr[:], in_=brfr_d[:, :])
            attfr = cst.tile([P, DF], F32, tag="attfr", name="atf_t")
            nc.sync.dma_start(out=attfr[:], in_=attfr_d[:, :])
            wef = cst.tile([1, DF], BF, tag="wef", name="wef_t")
            nc.sync.dma_start(out=wef[:], in_=wef_d[:, :])
            biafr = cst.tile([P, P], F32, tag="biafr", name="biaf_t")
            nc.sync.dma_start(out=biafr[:], in_=biafr_d[:, :])

            with tc.tile_pool(name="psf", bufs=1, space="PSUM") as ps:
                # budget: ep(1 x 512f32=1 bank... [P,DF] = 2KB = 1 bank) x2
                #         + nm0..3 (4) + dnm(1) + pt(1) = 8
                def ep_t():
                    return ps.tile([P, DF], F32, space="PSUM", tag="fep",
                                   bufs=2, name="fep")

                def pt_t():
                    return ps.tile([P, P], F32, space="PSUM", tag="fpt",
                                   name="fpt")

                for w in range(NW):
                    tp = ps.tile([P, P], BF, space="PSUM", tag="fpt",
                                 name="ftpb")
                    nc.tensor.transpose(out=tp[:], in_=h_cur[:, w, :],
                                        identity=identb[:])
                    nc.vector.tensor_copy(out=hT[:, w, :], in_=tp[:])
                    xp = ep_t()
                    nc.tensor.matmul(out=xp[:], lhsT=hT[:, w, :], rhs=wlf[:],
                                     start=True, stop=True)
                    xs = wsp.tile([P, DF], BF, tag="fxs", bufs=1, name="fxs")
                    nc.vector.tensor_tensor(out=xs[:], in0=xp[:], in1=blfr[:],
                                            op=OP.add)
                    nc.sync.dma_start(out=xlf_b[w * P:(w + 1) * P, :],
                                      in_=xs[:])

                nc.gpsimd.collective_compute(
                    "AllGather", OP.bypass, replica_groups=groups,
                    ins=[xlf_b.opt()], outs=[xlf_full.opt()])

                for w in range(NW):
                    T = int(Tw[w])
                    t0 = int(toff[w])
                    xrp = ep_t()
                    nc.tensor.matmul(out=xrp[:], lhsT=hT[:, w, :], rhs=wrf[:],
                                     start=True, stop=True)
                    xr = wsp.tile([P, DF], BF, tag="fxr", bufs=1, name="fxr")
                    nc.vector.tensor_tensor(out=xr[:], in0=xrp[:], in1=brfr[:],
                                            op=OP.add)

                    fnm = ps.tile([P, DF], F32, space="PSUM", tag="fnm",
                                  name="fnm")
                    dnm = ps.tile([P, HEADS], F32, space="PSUM", tag="fdnm",
                                  name="fdnm")

                    for t in range(T):
                        gq = gpo.tile([P, DF], BF, tag="fgq", bufs=4, name="fgq")
                        nc.gpsimd.indirect_dma_start(
                            out=gq[:], out_offset=None, in_=xlf_full.opt(),
                            in_offset=bass.IndirectOffsetOnAxis(
                                ap=gidx_s[:, t0 + t:t0 + t + 1], axis=0))
                        dstf_s = wsp.tile([1, 4 * P], F32, tag="dstf",
                                          name="dsf2")
                        nc.sync.dma_start(
                            out=dstf_s[0:1, :P],
                            in_=dstf_d[0:1, (t0 + t) * P:(t0 + t + 1) * P])
                        ew_s = wsp.tile([1, 4 * P], BF, tag="ews", name="ews2")
                        nc.sync.dma_start(
                            out=ew_s[0:1, :P],
                            in_=ew_d[0:1, (t0 + t) * P:(t0 + t + 1) * P])
                        db = pt_t()
                        nc.tensor.matmul(out=db[:], lhsT=ones1[:],
                                         rhs=dstf_s[0:1, :P],
                                         start=True, stop=True)
                        S = wsp.tile([P, P], BF, tag="S", name="Sf")
                        nc.vector.tensor_tensor(
                            out=S[:], in0=db[:],
                            in1=iota_cf[:, 0:1].to_broadcast([P, P]),
                            op=OP.is_equal)
                        ST = wsp.tile([P, P], BF, tag="STf", name="STf")
                        nc.vector.tensor_tensor(
                            out=ST[:], in0=iota_row[:],
                            in1=dsti_s[:, t0 + t:t0 + t + 1].to_broadcast(
                                [P, P]),
                            op=OP.is_equal)
                        ep = ep_t()
                        nc.tensor.matmul(out=ep[:], lhsT=identb[:], rhs=gq[:],
                                         start=True, stop=False)
                        nc.tensor.matmul(out=ep[:], lhsT=S[:], rhs=xr[:],
                                         start=False, stop=False)
                        nc.tensor.matmul(out=ep[:],
                                         lhsT=ew_s[0:1, :P],
                                         rhs=wef[:], start=False, stop=True)
                        ea = wsp.tile([P, DF], F32, tag="fea", bufs=1, name="fea")
                        leaky(ea[:], ep[:], [P, DF])
                        lg = wsp.tile([P, DF], F32, tag="flg", bufs=1, name="flg")
                        nc.vector.tensor_tensor(out=lg[:], in0=ea[:],
                                                in1=attfr[:], op=OP.mult)
                        lgr = wsp.tile([P, HEADS], F32, tag="flgr",
                                       name="flgr")
                        nc.vector.tensor_reduce(
                            out=lgr[:],
                            in_=lg[:].rearrange("p (h c) -> p h c", h=HEADS,
                                                c=P),
                            axis=AX.X, op=OP.add)
                        wq = wsp.tile([P, HEADS], BF, tag="fwq", name="fwq")
                        nc.scalar.activation(out=wq[:], in_=lgr[:], func=AF.Exp)
                        mm = wsp.tile([P, HEADS, P], BF, tag="fmm", bufs=1,
                                      name="fmm")
                        nc.vector.tensor_tensor(
                            out=mm[:],
                            in0=gq[:].rearrange("p (h c) -> p h c", h=HEADS,
                                                c=P),
                            in1=wq[:, :, None].to_broadcast([P, HEADS, P]),
                            op=OP.mult)
                        nc.tensor.matmul(
                            out=fnm[:], lhsT=ST[:],
                            rhs=mm[:].rearrange("p h c -> p (h c)"),
                            start=(t == 0), stop=(t == T - 1))
                        nc.tensor.matmul(out=dnm[:], lhsT=ST[:], rhs=wq[:],
                                         start=(t == 0), stop=(t == T - 1))

                    # flush: out = bias + sum_h numer[n,h,:]*(0.25/denom[n,h])
                    rd = wsp.tile([P, HEADS], F32, tag="rd", name="rdf")
                    nc.vector.tensor_scalar(out=rd[:], in0=dnm[:],
                                            scalar1=1e-16, scalar2=None,
                                            op0=OP.add)
                    nc.vector.reciprocal(out=rd[:], in_=rd[:])
                    nc.vector.tensor_scalar(out=rd[:], in0=rd[:],
                                            scalar1=1.0 / HEADS, scalar2=None,
                                            op0=OP.mult)
                    sc = wsp.tile([P, HEADS, P], F32, tag="sc", bufs=1,
                                  name="sc")
                    nc.vector.tensor_tensor(
                        out=sc[:],
                        in0=fnm[:].rearrange("p (h c) -> p h c", h=HEADS, c=P),
                        in1=rd[:, :, None].to_broadcast([P, HEADS, P]),
                        op=OP.mult)
                    acc = wsp.tile([P, P], F32, tag="acc", name="acc")
                    nc.vector.tensor_reduce(
                        out=acc[:], in_=sc[:].rearrange("p h c -> p c h"),
                        axis=AX.X, op=OP.add)
                    nc.vector.tensor_tensor(out=acc[:], in0=acc[:],
                                            in1=biafr[:], op=OP.add)
                    nc.sync.dma_start(out=out_d[w * P:(w + 1) * P, :],
                                      in_=acc[:])

        # ---- the 4 layers ----
        hidden_layer(0, h_a, h_a, add_resid=False)
        hidden_layer(1, h_a, h_a, add_resid=True)
        hidden_layer(2, h_a, h_a, add_resid=False)
        final_layer(h_a)

    nc.compile()
    return nc


# ----------------------------------------------------------------------------
# Host-side driver
# ----------------------------------------------------------------------------

def _rep(v):
    v = np.asarray(v, np.float32).reshape(-1)
    return np.broadcast_to(v, (P, v.shape[0])).copy()


def make_in_maps(meta, inputs):
    NPAD, TT = meta["NPAD"], meta["TT"]
    lo, hi = meta["lo"], meta["hi"]
    x = np.asarray(inputs["x"], np.float32)
    resid = np.asarray(inputs["residual"], np.float32)
    ew = np.asarray(inputs["edge_weight"], np.float32)

    att = np.asarray(inputs["att"], np.float32)        # (3, H, C)
    attf = np.asarray(inputs["att_f"], np.float32)     # (H, DOUT)

    common = dict(
        invd=meta["invd"].astype(np.float32),
        sel4=meta["sel4"],
        wl=np.asarray(inputs["Wl"], np.float32).astype(ml_dtypes.bfloat16),
        wr=np.asarray(inputs["Wr"], np.float32).astype(ml_dtypes.bfloat16),
        blr=np.stack([_rep(inputs["bl"][i]) for i in range(3)]),
        brr=np.stack([_rep(inputs["br"][i]) for i in range(3)]),
        attr=np.stack([_rep(att[i]) for i in range(3)]),
        we=np.asarray(inputs["We"], np.float32).astype(ml_dtypes.bfloat16),
        lnw=np.stack([_rep(inputs["ln_w"][i]) for i in range(3)]),
        lnb=np.stack([_rep(inputs["ln_b"][i]) for i in range(3)]),
        bia=np.stack([_rep(inputs["bias"][i]) for i in range(3)]),
        wlf=np.asarray(inputs["Wl_f"], np.float32).astype(ml_dtypes.bfloat16),
        wrf=np.asarray(inputs["Wr_f"], np.float32).astype(ml_dtypes.bfloat16),
        blfr=_rep(inputs["bl_f"]),
        brfr=_rep(inputs["br_f"]),
        attfr=_rep(attf),
        wef=np.asarray(inputs["We_f"], np.float32).reshape(1, DF).astype(ml_dtypes.bfloat16),
        biafr=_rep(inputs["bias_f"]),
    )

    in_maps = []
    for c in range(NCORES):
        n = int(hi[c] - lo[c])
        h0s = np.zeros((NPAD, P), ml_dtypes.bfloat16)
        h0s[:n] = x[lo[c]:hi[c]].astype(ml_dtypes.bfloat16)
        rss = np.zeros((NPAD, P), np.float32)
        rss[:n] = resid[lo[c]:hi[c]]
        ewc = np.zeros(TT * P, np.float32)
        m = meta["ewsl"][c] >= 0
        ewc[m] = ew[meta["ewsl"][c][m]]
        in_maps.append(dict(
            h0s=h0s, rs=rss,
            gidx=meta["gidx"][c], dsti=meta["dsti"][c],
            dstf=meta["dstf"][c].reshape(1, -1),
            ew=ewc.reshape(1, -1).astype(ml_dtypes.bfloat16),
            gmat=meta["gmat"][c], gmatT=meta["gmatT"][c],
            **common))
    return in_maps


def assemble(meta, results):
    N = meta["N"]
    lo, hi = meta["lo"], meta["hi"]
    out = np.zeros((N, P), np.float32)
    for c in range(NCORES):
        n = int(hi[c] - lo[c])
        out[lo[c]:hi[c]] = results[c]["out"][:n]
    return out


_CACHE = {}


def kernel(**inputs):
    ei = np.asarray(inputs["edge_index"])
    bt = np.asarray(inputs["batch"])
    key = (ei.shape, bt.shape, hash(ei.tobytes()), hash(bt.tobytes()))
    if key not in _CACHE:
        meta = build_meta(ei, bt)
        nc = build_program(meta)
        _CACHE[key] = (meta, nc)
    meta, nc = _CACHE[key]
    in_maps = make_in_maps(meta, inputs)
    res = run_bass_kernel_spmd(nc, in_maps, list(range(NCORES)))
    return assemble(meta, res.results)

